# revision 6
# baseline (speedup 1.0000x reference)
"""CapsuleConv2d (3x3, s1, p1, L_in=4, L_out=8, 3 routing iters) on 8 trn2 cores.

Sharding: data-parallel over (N=4 images) x (2 half-images of 28 rows) = 8
shards, one per core.

Dispatch is latency-optimized for the axon tunnel (wall-clock is dominated by
host<->device RPC legs, not device compute):
  - inputs shipped compact in fp16: per-core padded x slice (111KB) + the raw
    37KB weight tensor + a 512B block-diag mask; the block-diagonal matmul
    operand (wmm) and the uniform-vote operand (wsum) are expanded ON DEVICE
    (one DVE multiply + one ACT scale), instead of shipping 664KB of
    host-expanded fp32 weights per core
  - output shipped int8 (102KB/core): |v| < 1 is guaranteed by squash, so the
    final squash folds a x127 scale into phi and writes int8 directly; the
    host divides by 127 (quantization adds ~0.004 abs err vs the 2e-2 gate)
  - the work is split into FOUR independent 2-core dispatches driven by four
    threads; their h2d/exec/d2h RPC chains overlap on the tunnel (measured
    ~20% faster than one 8-core dispatch, whose legs serialize)
  - each jitted shard_map callable is built/compiled ONCE and cached; a call
    issues h2d + exec + d2h fully async and blocks only on the final host
    copies; each thread also builds its own input block and assembles its own
    output chunks, overlapping host prep with the other threads' RPCs

Device kernel (per core, 1568 positions as 14 tiles of 2 rows):
  - PE: priors u via block-diag matmuls (fp16 in, fp32 PSUM); the uniform
    first vote s0 folded into a second accumulating matmul
  - DVE: products, segmented reductions, softmax pieces, squash
  - ACT: PSUM->SBUF copies, exp, sqrt
Per-position free-dim layout for priors u[c,m,k,g]: idx = c*576 + m*72 + k*8 + g
(c = out-capsule 8, m = out-length 8, k = kernel offset 9, g = in-capsule 8).

Engine ISA instructions carry at most one semaphore wait in hardware; building
on bacc.Bacc (not bass.Bass) runs generate_event_semaphores at finalize, which
legalizes the multi-waits Tile emits for cross-engine dependencies.
"""

import numpy as np

import concourse.bass as bass
import concourse.mybir as mybir
import concourse.tile as tile
from concourse import bacc

FP32 = mybir.dt.float32
FP16 = mybir.dt.float16
INT8 = mybir.dt.int8
VSCALE = 127.0   # |v| < 1 always (squash: ||v|| = ||s||^2/(1+||s||^2) < 1)
AF = mybir.ActivationFunctionType
MULT = mybir.AluOpType.mult

KK, GI, GO, LI, LO = 9, 8, 8, 4, 8
HO = WO = 56
ROWS = 28            # output rows per core
SH, SW = ROWS + 2, WO + 2   # 30 x 58 padded input slice per core
TP = 114             # 2 output rows + 2 junk pad positions per tile
NT = 14              # tiles per core (2 rows each)
CM = GO * LO         # 64
CKG = GO * KK * GI   # 576 (c,k,g)
UF = GO * LO * KK * GI  # 4608 (c,m,k,g)
WC = KK * CM         # 576 compact weight columns: (k,c,m)

# free-dim strides in u
SC, SM, SK, SG = 576, 72, 8, 1

XCOLS = SH * SW                  # fp16 x columns per core
INF = XCOLS + WC + GI            # fused input columns: x | wc | mask

N_WAY = 4                        # parallel dispatches
CORES_PER = 2                    # cores per dispatch


def _v(a, dims):
    """Re-view an AP (taken at a tile's origin) with explicit free [step,count] dims."""
    return bass.AP(a.tensor, a.offset, [list(a.ap[0])] + [list(d) for d in dims])


def build_program():
    nc = bacc.Bacc()
    # single fused input (one DMA, one semaphore -> LDWEIGHTS can encode the wait)
    inp = nc.dram_tensor("inp", [32, INF], FP16, kind="ExternalInput")
    out = nc.dram_tensor("out", [NT * TP, CM], INT8, kind="ExternalOutput")

    with tile.TileContext(nc) as tc:
        with (
            tc.tile_pool(name="singles", bufs=1) as singles,
            tc.tile_pool(name="upool", bufs=2) as upool,
            tc.tile_pool(name="ttpool", bufs=4) as ttpool,
            tc.tile_pool(name="mid", bufs=4) as mid,
            tc.tile_pool(name="tiny", bufs=3) as tiny,
            tc.tile_pool(name="vout", bufs=3) as vout,
            tc.tile_pool(name="pu", bufs=3, space="PSUM") as pupool,
            tc.tile_pool(name="ps0", bufs=2, space="PSUM") as ps0pool,
        ):
            inp_sb = singles.tile([32, INF], FP16)
            nc.sync.dma_start(out=inp_sb[:], in_=inp[:])
            xs_flat = inp_sb[:, :XCOLS]
            wc_sb = inp_sb[:, XCOLS:XCOLS + WC]  # [32, (k,c,m)]
            mask_sb = inp_sb[:, XCOLS + WC:]     # [32, g2]: 1 iff g2 == p//4

            # ---- on-device weight expansion ----
            # wsum[(g,l), k*64 + c*8 + m] = wc/9  (same layout as wc)
            wsum_sb = singles.tile([32, WC], FP16)
            nc.scalar.mul(wsum_sb[:], wc_sb, 1.0 / KK)
            # wmm[(g,l), k*512 + (c*8+m)*8 + g2] = wc[(g,l), k*64+c*8+m] * mask[g2]
            wmm_sb = singles.tile([32, KK * 512], FP16)
            nc.vector.tensor_tensor(
                _v(wmm_sb[:], [[512, KK], [8, CM], [1, GI]]),
                _v(wc_sb, [[CM, KK], [1, CM], [0, GI]]),
                _v(mask_sb, [[0, KK], [0, CM], [1, GI]]),
                op=MULT)

            for t in range(NT):
                h0 = 2 * t
                # ---- priors: u[pos; c,m,k,g] and s0[pos; c,m] on PE ----
                u = upool.tile([TP, UF], FP32)
                ps0 = ps0pool.tile([TP, CM], FP32)
                for k in range(KK):
                    di, dj = k // 3, k % 3
                    # flat 114-run covering 2 rows of 56 (+2 junk at 56,57):
                    # LDWEIGHTS needs a single-free-dim AP
                    o = (h0 + di) * SW + dj
                    lhsT = xs_flat[:, o:o + TP]  # [32, 114] fp16
                    pu = pupool.tile([TP, 512], FP32)
                    nc.tensor.matmul(pu[:], lhsT, wmm_sb[:, k * 512:(k + 1) * 512],
                                     start=True, stop=True)
                    nc.tensor.matmul(ps0[:], lhsT, wsum_sb[:, k * CM:(k + 1) * CM],
                                     start=(k == 0), stop=(k == KK - 1))
                    # psum (c,m,g) -> sbuf u[:, c,m,k=k,g]  (strided write, ACT)
                    u4 = _v(u[:], [[SC, GO], [SM, LO], [SK, KK], [SG, GI]])
                    nc.scalar.copy(out=u4[:, :, :, k, :], in_=pu[:])

                # ---- routing ----
                def squash(s_ap, vdst, scale=None):
                    sq = tiny.tile([TP, CM], FP32, tag="sq")
                    nc.vector.tensor_mul(sq[:], s_ap, s_ap)
                    n2 = tiny.tile([TP, GO], FP32, tag="n2")
                    nc.vector.reduce_sum(n2[:], _v(sq[:], [[LO, GO], [1, LO]]),
                                         axis=mybir.AxisListType.X)
                    rt = tiny.tile([TP, GO], FP32, tag="rt")
                    nc.scalar.activation(rt[:], n2[:], AF.Sqrt)
                    n2p1 = tiny.tile([TP, GO], FP32, tag="n2p1")
                    nc.scalar.add(n2p1[:], n2[:], 1.0)
                    inv = tiny.tile([TP, GO], FP32, tag="inv")
                    nc.vector.reciprocal(inv[:], n2p1[:])
                    phi = tiny.tile([TP, GO], FP32, tag="phi")
                    nc.vector.tensor_mul(phi[:], rt[:], inv[:])
                    if scale is not None:
                        phis = tiny.tile([TP, GO], FP32, tag="phis")
                        nc.scalar.mul(phis[:], phi[:], scale)
                        phi = phis
                    # v = s * phi (phi broadcast over m)
                    return nc.vector.tensor_tensor(
                        _v(vdst[:], [[LO, GO], [1, LO]]),
                        bass.AP(s_ap.tensor, s_ap.offset,
                                [list(s_ap.ap[0]), [LO, GO], [1, LO]]),
                        _v(phi[:], [[1, GO], [0, LO]]),
                        op=MULT)

                s0 = tiny.tile([TP, CM], FP32, tag="s0")
                nc.scalar.copy(out=s0[:], in_=ps0[:])
                v = vout.tile([TP, CM], FP32, tag="v")
                squash(s0[:], v)

                b_prev = None
                for r in (1, 2):
                    # tt = u * v  (v[c,m] broadcast over k,g)
                    tt = ttpool.tile([TP, UF], FP32, tag="tt")
                    nc.vector.tensor_tensor(
                        _v(tt[:], [[SC, GO], [SM, LO], [1, KK * GI]]),
                        _v(u[:], [[SC, GO], [SM, LO], [1, KK * GI]]),
                        _v(v[:], [[LO, GO], [1, LO], [0, KK * GI]]),
                        op=MULT)
                    # b = sum_m tt  -> [pos; c,k,g]
                    b = mid.tile([TP, CKG], FP32, tag="b")
                    nc.vector.reduce_sum(
                        b[:], _v(tt[:], [[SC, GO], [SK, KK], [SG, GI], [SM, LO]]),
                        axis=mybir.AxisListType.X)
                    if b_prev is not None:
                        nc.vector.tensor_add(b[:], b[:], b_prev[:])
                    b_prev = b
                    # softmax over k (segments of the c,k,g layout)
                    e = mid.tile([TP, CKG], FP32, tag="e")
                    nc.scalar.activation(e[:], b[:], AF.Exp)
                    ssum = tiny.tile([TP, CM], FP32, tag="ssum")
                    nc.vector.reduce_sum(
                        ssum[:], _v(e[:], [[KK * GI, GO], [SG, GI], [SK, KK]]),
                        axis=mybir.AxisListType.X)
                    invs = tiny.tile([TP, CM], FP32, tag="invs")
                    nc.vector.reciprocal(invs[:], ssum[:])
                    p = mid.tile([TP, CKG], FP32, tag="p")
                    nc.vector.tensor_tensor(
                        _v(p[:], [[KK * GI, GO], [SK, KK], [SG, GI]]),
                        _v(e[:], [[KK * GI, GO], [SK, KK], [SG, GI]]),
                        _v(invs[:], [[GI, GO], [0, KK], [1, GI]]),
                        op=MULT)
                    # tt2 = p * u ; s = sum_{k,g} tt2
                    tt2 = ttpool.tile([TP, UF], FP32, tag="tt")
                    nc.vector.tensor_tensor(
                        _v(tt2[:], [[SC, GO], [SM, LO], [SK, KK], [SG, GI]]),
                        _v(u[:], [[SC, GO], [SM, LO], [SK, KK], [SG, GI]]),
                        _v(p[:], [[KK * GI, GO], [0, LO], [SK, KK], [SG, GI]]),
                        op=MULT)
                    s = tiny.tile([TP, CM], FP32, tag="s")
                    nc.vector.reduce_sum(
                        s[:], _v(tt2[:], [[SC, GO], [SM, LO], [SK, KK], [SG, GI]]),
                        axis=mybir.AxisListType.XY)
                    # final iteration writes scaled int8 directly (DMA'd out)
                    if r == 2:
                        v = vout.tile([TP, CM], INT8, tag="v8")
                        squash(s[:], v, scale=VSCALE)
                    else:
                        v = vout.tile([TP, CM], FP32, tag="v")
                        squash(s[:], v)

                nc.sync.dma_start(out=out[t * TP:(t + 1) * TP, :], in_=v[:])
    return nc


# ---------------- host side ----------------

_STATE = None

# (image, top row) of each 28-row chunk; chunk i runs on global core i
_CHUNKS = [(n, h0) for n in range(4) for h0 in (0, ROWS)]


def _get_state():
    """Build the program and the cached jitted callables (one per 2-core
    dispatch group) once."""
    global _STATE
    if _STATE is None:
        import jax
        import concurrent.futures as cf
        from jax.sharding import Mesh, PartitionSpec
        from jax.experimental.shard_map import shard_map
        from concourse.bass2jax import (_bass_exec_p, install_neuronx_cc_hook,
                                        partition_id_tensor)

        nc = build_program()
        nc.finalize()
        install_neuronx_cc_hook()

        partition_name = (nc.partition_id_tensor.name
                          if nc.partition_id_tensor else None)
        in_names, out_names, out_avals = [], [], []
        for alloc in nc.m.functions[0].allocations:
            if not isinstance(alloc, mybir.MemoryLocationSet):
                continue
            name = alloc.memorylocations[0].name
            if alloc.kind == "ExternalInput":
                if name != partition_name:
                    in_names.append(name)
            elif alloc.kind == "ExternalOutput":
                out_names.append(name)
                out_avals.append(jax.core.ShapedArray(
                    tuple(alloc.tensor_shape), mybir.dt.np(alloc.dtype)))
        all_names = list(in_names)
        if partition_name is not None:
            all_names.append(partition_name)

        def _body(*args):
            operands = list(args)
            if partition_name is not None:
                operands.append(partition_id_tensor())
            return tuple(_bass_exec_p.bind(
                *operands,
                out_avals=tuple(out_avals), in_names=tuple(all_names),
                out_names=tuple(out_names), lowering_input_output_aliases=(),
                sim_require_finite=True, sim_require_nnan=True, nc=nc))

        devices = jax.devices()[:N_WAY * CORES_PER]
        dispatches = []
        for g in range(N_WAY):
            mesh = Mesh(np.asarray(devices[g * CORES_PER:(g + 1) * CORES_PER]),
                        ("core",))
            dispatches.append(jax.jit(
                shard_map(_body, mesh=mesh,
                          in_specs=(PartitionSpec("core"),) * len(in_names),
                          out_specs=(PartitionSpec("core"),) * len(out_names),
                          check_rep=False),
                keep_unused=True))
        pool = cf.ThreadPoolExecutor(N_WAY)
        _STATE = (dispatches, pool)
    return _STATE


def _run_group(dispatch, xp, wc, mask, cores, out_buf):
    """Build this group's input block, dispatch it, fetch, and assemble its
    chunks into out_buf (all inside the worker thread so host work overlaps
    the other groups' RPC legs)."""
    blk = np.empty((len(cores) * 32, INF), np.float16)
    for i, core in enumerate(cores):
        n, h0 = _CHUNKS[core]
        rows = blk[i * 32:(i + 1) * 32]
        rows[:, :XCOLS] = xp[n, :, h0:h0 + SH, :].reshape(32, XCOLS)
        rows[:, XCOLS:XCOLS + WC] = wc
        rows[:, XCOLS + WC:] = mask
    out = dispatch(blk)[0]
    shards = sorted(out.addressable_shards, key=lambda s: s.index[0].start or 0)
    for s in shards:
        s.data.copy_to_host_async()
    for i, core in enumerate(cores):
        o = np.asarray(shards[i].data, np.float32).reshape(NT, TP, CM)
        o *= 1.0 / VSCALE
        # TP=114 run: [0:56] = row 0, [58:114] = row 1, 56/57 junk
        o = np.stack([o[:, :WO], o[:, SW:SW + WO]], axis=1).reshape(ROWS, WO, CM)
        n, h0 = _CHUNKS[core]
        out_buf[n, :, h0:h0 + ROWS, :] = np.transpose(o, (2, 0, 1))


def kernel(x, weight):
    dispatches, pool = _get_state()
    xp = np.pad(np.asarray(x).astype(np.float16),
                ((0, 0), (0, 0), (1, 1), (1, 1)))
    wr = np.asarray(weight, np.float32).reshape(GO, GI, KK, LI, LO)
    # wc[(g,l), k*64 + c*8 + m] = wr[c,g,k,l,m]
    wc = np.transpose(wr, (1, 3, 2, 0, 4)).reshape(32, WC).astype(np.float16)
    mask = np.zeros((32, GI), np.float16)
    mask[np.arange(32), np.arange(32) // LI] = 1.0
    out_buf = np.empty((4, GO * LO, HO, WO), np.float32)
    futs = [pool.submit(_run_group, dispatches[g], xp, wc, mask,
                        list(range(g * CORES_PER, (g + 1) * CORES_PER)), out_buf)
            for g in range(N_WAY)]
    for f in futs:
        f.result()
    return out_buf


# revision 8
# speedup vs baseline: 1.2400x; 1.2400x over previous
"""CapsuleConv2d (3x3, s1, p1, L_in=4, L_out=8, 3 routing iters) on 8 trn2 cores.

Sharding: data-parallel over (N=4 images) x (2 half-images of 28 rows) = 8
shards, one per core.

Dispatch is latency-optimized for the axon tunnel (wall-clock is dominated by
host<->device RPC legs, not device compute):
  - inputs shipped compact in fp16: per-core padded x slice (111KB) + the raw
    37KB weight tensor + a 512B block-diag mask; the block-diagonal matmul
    operand (wmm) and the uniform-vote operand (wsum) are expanded ON DEVICE
    (one DVE multiply + one ACT scale), instead of shipping 664KB of
    host-expanded fp32 weights per core
  - output shipped int8 (102KB/core): |v| < 1 is guaranteed by squash, so the
    final squash folds a x127 scale into phi and writes int8 directly; the
    host divides by 127 (quantization adds ~0.004 abs err vs the 2e-2 gate)
  - the work is split into EIGHT independent single-core dispatches driven by
    eight threads; their h2d/exec/d2h RPC chains overlap on the tunnel
    (measured ~20% faster than one 8-core dispatch, whose legs serialize)
  - each jitted shard_map callable is built/compiled ONCE and cached; a call
    issues h2d + exec + d2h fully async and blocks only on the final host
    copies; each thread also builds its own input block and assembles its own
    output chunks, overlapping host prep with the other threads' RPCs

Device kernel (per core, 1568 positions as 14 tiles of 2 rows):
  - PE: priors u via block-diag matmuls (fp16 in, fp32 PSUM); the uniform
    first vote s0 folded into a second accumulating matmul
  - DVE: products, segmented reductions, softmax pieces, squash
  - ACT: PSUM->SBUF copies, exp, sqrt
Per-position free-dim layout for priors u[c,m,k,g]: idx = c*576 + m*72 + k*8 + g
(c = out-capsule 8, m = out-length 8, k = kernel offset 9, g = in-capsule 8).

Engine ISA instructions carry at most one semaphore wait in hardware; building
on bacc.Bacc (not bass.Bass) runs generate_event_semaphores at finalize, which
legalizes the multi-waits Tile emits for cross-engine dependencies.
"""

import numpy as np

import concourse.bass as bass
import concourse.mybir as mybir
import concourse.tile as tile
from concourse import bacc

FP32 = mybir.dt.float32
FP16 = mybir.dt.float16
INT8 = mybir.dt.int8
VSCALE = 127.0   # |v| < 1 always (squash: ||v|| = ||s||^2/(1+||s||^2) < 1)
AF = mybir.ActivationFunctionType
MULT = mybir.AluOpType.mult

KK, GI, GO, LI, LO = 9, 8, 8, 4, 8
HO = WO = 56
ROWS = 28            # output rows per core
SH, SW = ROWS + 2, WO + 2   # 30 x 58 padded input slice per core
TP = 114             # 2 output rows + 2 junk pad positions per tile
NT = 14              # tiles per core (2 rows each)
CM = GO * LO         # 64
CKG = GO * KK * GI   # 576 (c,k,g)
UF = GO * LO * KK * GI  # 4608 (c,m,k,g)
WC = KK * CM         # 576 compact weight columns: (k,c,m)

# free-dim strides in u
SC, SM, SK, SG = 576, 72, 8, 1

XCOLS = SH * SW                  # fp16 x columns per core
INF = XCOLS + WC + GI            # fused input columns: x | wc | mask

N_WAY = 8                        # parallel dispatches
CORES_PER = 1                    # cores per dispatch


def _v(a, dims):
    """Re-view an AP (taken at a tile's origin) with explicit free [step,count] dims."""
    return bass.AP(a.tensor, a.offset, [list(a.ap[0])] + [list(d) for d in dims])


def build_program():
    nc = bacc.Bacc()
    # single fused input (one DMA, one semaphore -> LDWEIGHTS can encode the wait)
    inp = nc.dram_tensor("inp", [32, INF], FP16, kind="ExternalInput")
    out = nc.dram_tensor("out", [NT * TP, CM], INT8, kind="ExternalOutput")

    with tile.TileContext(nc) as tc:
        with (
            tc.tile_pool(name="singles", bufs=1) as singles,
            tc.tile_pool(name="upool", bufs=2) as upool,
            tc.tile_pool(name="ttpool", bufs=4) as ttpool,
            tc.tile_pool(name="mid", bufs=4) as mid,
            tc.tile_pool(name="tiny", bufs=3) as tiny,
            tc.tile_pool(name="vout", bufs=3) as vout,
            tc.tile_pool(name="pu", bufs=3, space="PSUM") as pupool,
            tc.tile_pool(name="ps0", bufs=2, space="PSUM") as ps0pool,
        ):
            inp_sb = singles.tile([32, INF], FP16)
            nc.sync.dma_start(out=inp_sb[:], in_=inp[:])
            xs_flat = inp_sb[:, :XCOLS]
            wc_sb = inp_sb[:, XCOLS:XCOLS + WC]  # [32, (k,c,m)]
            mask_sb = inp_sb[:, XCOLS + WC:]     # [32, g2]: 1 iff g2 == p//4

            # ---- on-device weight expansion ----
            # wsum[(g,l), k*64 + c*8 + m] = wc/9  (same layout as wc)
            wsum_sb = singles.tile([32, WC], FP16)
            nc.scalar.mul(wsum_sb[:], wc_sb, 1.0 / KK)
            # wmm[(g,l), k*512 + (c*8+m)*8 + g2] = wc[(g,l), k*64+c*8+m] * mask[g2]
            wmm_sb = singles.tile([32, KK * 512], FP16)
            nc.vector.tensor_tensor(
                _v(wmm_sb[:], [[512, KK], [8, CM], [1, GI]]),
                _v(wc_sb, [[CM, KK], [1, CM], [0, GI]]),
                _v(mask_sb, [[0, KK], [0, CM], [1, GI]]),
                op=MULT)

            for t in range(NT):
                h0 = 2 * t
                # ---- priors: u[pos; c,m,k,g] and s0[pos; c,m] on PE ----
                u = upool.tile([TP, UF], FP32)
                ps0 = ps0pool.tile([TP, CM], FP32)
                for k in range(KK):
                    di, dj = k // 3, k % 3
                    # flat 114-run covering 2 rows of 56 (+2 junk at 56,57):
                    # LDWEIGHTS needs a single-free-dim AP
                    o = (h0 + di) * SW + dj
                    lhsT = xs_flat[:, o:o + TP]  # [32, 114] fp16
                    pu = pupool.tile([TP, 512], FP32)
                    nc.tensor.matmul(pu[:], lhsT, wmm_sb[:, k * 512:(k + 1) * 512],
                                     start=True, stop=True)
                    nc.tensor.matmul(ps0[:], lhsT, wsum_sb[:, k * CM:(k + 1) * CM],
                                     start=(k == 0), stop=(k == KK - 1))
                    # psum (c,m,g) -> sbuf u[:, c,m,k=k,g]  (strided write, ACT)
                    u4 = _v(u[:], [[SC, GO], [SM, LO], [SK, KK], [SG, GI]])
                    nc.scalar.copy(out=u4[:, :, :, k, :], in_=pu[:])

                # ---- routing ----
                def squash(s_ap, vdst, scale=None):
                    sq = tiny.tile([TP, CM], FP32, tag="sq")
                    nc.vector.tensor_mul(sq[:], s_ap, s_ap)
                    n2 = tiny.tile([TP, GO], FP32, tag="n2")
                    nc.vector.reduce_sum(n2[:], _v(sq[:], [[LO, GO], [1, LO]]),
                                         axis=mybir.AxisListType.X)
                    rt = tiny.tile([TP, GO], FP32, tag="rt")
                    nc.scalar.activation(rt[:], n2[:], AF.Sqrt)
                    n2p1 = tiny.tile([TP, GO], FP32, tag="n2p1")
                    nc.scalar.add(n2p1[:], n2[:], 1.0)
                    inv = tiny.tile([TP, GO], FP32, tag="inv")
                    nc.vector.reciprocal(inv[:], n2p1[:])
                    phi = tiny.tile([TP, GO], FP32, tag="phi")
                    nc.vector.tensor_mul(phi[:], rt[:], inv[:])
                    if scale is not None:
                        phis = tiny.tile([TP, GO], FP32, tag="phis")
                        nc.scalar.mul(phis[:], phi[:], scale)
                        phi = phis
                    # v = s * phi (phi broadcast over m)
                    return nc.vector.tensor_tensor(
                        _v(vdst[:], [[LO, GO], [1, LO]]),
                        bass.AP(s_ap.tensor, s_ap.offset,
                                [list(s_ap.ap[0]), [LO, GO], [1, LO]]),
                        _v(phi[:], [[1, GO], [0, LO]]),
                        op=MULT)

                s0 = tiny.tile([TP, CM], FP32, tag="s0")
                nc.scalar.copy(out=s0[:], in_=ps0[:])
                v = vout.tile([TP, CM], FP32, tag="v")
                squash(s0[:], v)

                b_prev = None
                for r in (1, 2):
                    # tt = u * v  (v[c,m] broadcast over k,g)
                    tt = ttpool.tile([TP, UF], FP32, tag="tt")
                    nc.vector.tensor_tensor(
                        _v(tt[:], [[SC, GO], [SM, LO], [1, KK * GI]]),
                        _v(u[:], [[SC, GO], [SM, LO], [1, KK * GI]]),
                        _v(v[:], [[LO, GO], [1, LO], [0, KK * GI]]),
                        op=MULT)
                    # b = sum_m tt  -> [pos; c,k,g]
                    b = mid.tile([TP, CKG], FP32, tag="b")
                    nc.vector.reduce_sum(
                        b[:], _v(tt[:], [[SC, GO], [SK, KK], [SG, GI], [SM, LO]]),
                        axis=mybir.AxisListType.X)
                    if b_prev is not None:
                        nc.vector.tensor_add(b[:], b[:], b_prev[:])
                    b_prev = b
                    # softmax over k (segments of the c,k,g layout)
                    e = mid.tile([TP, CKG], FP32, tag="e")
                    nc.scalar.activation(e[:], b[:], AF.Exp)
                    ssum = tiny.tile([TP, CM], FP32, tag="ssum")
                    nc.vector.reduce_sum(
                        ssum[:], _v(e[:], [[KK * GI, GO], [SG, GI], [SK, KK]]),
                        axis=mybir.AxisListType.X)
                    invs = tiny.tile([TP, CM], FP32, tag="invs")
                    nc.vector.reciprocal(invs[:], ssum[:])
                    p = mid.tile([TP, CKG], FP32, tag="p")
                    nc.vector.tensor_tensor(
                        _v(p[:], [[KK * GI, GO], [SK, KK], [SG, GI]]),
                        _v(e[:], [[KK * GI, GO], [SK, KK], [SG, GI]]),
                        _v(invs[:], [[GI, GO], [0, KK], [1, GI]]),
                        op=MULT)
                    # tt2 = p * u ; s = sum_{k,g} tt2
                    tt2 = ttpool.tile([TP, UF], FP32, tag="tt")
                    nc.vector.tensor_tensor(
                        _v(tt2[:], [[SC, GO], [SM, LO], [SK, KK], [SG, GI]]),
                        _v(u[:], [[SC, GO], [SM, LO], [SK, KK], [SG, GI]]),
                        _v(p[:], [[KK * GI, GO], [0, LO], [SK, KK], [SG, GI]]),
                        op=MULT)
                    s = tiny.tile([TP, CM], FP32, tag="s")
                    nc.vector.reduce_sum(
                        s[:], _v(tt2[:], [[SC, GO], [SM, LO], [SK, KK], [SG, GI]]),
                        axis=mybir.AxisListType.XY)
                    # final iteration writes scaled int8 directly (DMA'd out)
                    if r == 2:
                        v = vout.tile([TP, CM], INT8, tag="v8")
                        squash(s[:], v, scale=VSCALE)
                    else:
                        v = vout.tile([TP, CM], FP32, tag="v")
                        squash(s[:], v)

                nc.sync.dma_start(out=out[t * TP:(t + 1) * TP, :], in_=v[:])
    return nc


# ---------------- host side ----------------

_STATE = None

# (image, top row) of each 28-row chunk; chunk i runs on global core i
_CHUNKS = [(n, h0) for n in range(4) for h0 in (0, ROWS)]


def _get_state():
    """Build the program and the cached jitted callables (one per 2-core
    dispatch group) once."""
    global _STATE
    if _STATE is None:
        import jax
        import concurrent.futures as cf
        from jax.sharding import Mesh, PartitionSpec
        from jax.experimental.shard_map import shard_map
        from concourse.bass2jax import (_bass_exec_p, install_neuronx_cc_hook,
                                        partition_id_tensor)

        nc = build_program()
        nc.finalize()
        install_neuronx_cc_hook()

        partition_name = (nc.partition_id_tensor.name
                          if nc.partition_id_tensor else None)
        in_names, out_names, out_avals = [], [], []
        for alloc in nc.m.functions[0].allocations:
            if not isinstance(alloc, mybir.MemoryLocationSet):
                continue
            name = alloc.memorylocations[0].name
            if alloc.kind == "ExternalInput":
                if name != partition_name:
                    in_names.append(name)
            elif alloc.kind == "ExternalOutput":
                out_names.append(name)
                out_avals.append(jax.core.ShapedArray(
                    tuple(alloc.tensor_shape), mybir.dt.np(alloc.dtype)))
        all_names = list(in_names)
        if partition_name is not None:
            all_names.append(partition_name)

        def _body(*args):
            operands = list(args)
            if partition_name is not None:
                operands.append(partition_id_tensor())
            return tuple(_bass_exec_p.bind(
                *operands,
                out_avals=tuple(out_avals), in_names=tuple(all_names),
                out_names=tuple(out_names), lowering_input_output_aliases=(),
                sim_require_finite=True, sim_require_nnan=True, nc=nc))

        devices = jax.devices()[:N_WAY * CORES_PER]
        dispatches = []
        for g in range(N_WAY):
            mesh = Mesh(np.asarray(devices[g * CORES_PER:(g + 1) * CORES_PER]),
                        ("core",))
            dispatches.append(jax.jit(
                shard_map(_body, mesh=mesh,
                          in_specs=(PartitionSpec("core"),) * len(in_names),
                          out_specs=(PartitionSpec("core"),) * len(out_names),
                          check_rep=False),
                keep_unused=True))
        pool = cf.ThreadPoolExecutor(N_WAY)
        _STATE = (dispatches, pool)
    return _STATE


def _run_group(dispatch, xp, wc, mask, cores, out_buf):
    """Build this group's input block, dispatch it, fetch, and assemble its
    chunks into out_buf (all inside the worker thread so host work overlaps
    the other groups' RPC legs)."""
    blk = np.empty((len(cores) * 32, INF), np.float16)
    for i, core in enumerate(cores):
        n, h0 = _CHUNKS[core]
        rows = blk[i * 32:(i + 1) * 32]
        rows[:, :XCOLS] = xp[n, :, h0:h0 + SH, :].reshape(32, XCOLS)
        rows[:, XCOLS:XCOLS + WC] = wc
        rows[:, XCOLS + WC:] = mask
    out = dispatch(blk)[0]
    shards = sorted(out.addressable_shards, key=lambda s: s.index[0].start or 0)
    for s in shards:
        s.data.copy_to_host_async()
    for i, core in enumerate(cores):
        o = np.asarray(shards[i].data, np.float32).reshape(NT, TP, CM)
        o *= 1.0 / VSCALE
        # TP=114 run: [0:56] = row 0, [58:114] = row 1, 56/57 junk
        o = np.stack([o[:, :WO], o[:, SW:SW + WO]], axis=1).reshape(ROWS, WO, CM)
        n, h0 = _CHUNKS[core]
        out_buf[n, :, h0:h0 + ROWS, :] = np.transpose(o, (2, 0, 1))


def kernel(x, weight):
    dispatches, pool = _get_state()
    xp = np.pad(np.asarray(x).astype(np.float16),
                ((0, 0), (0, 0), (1, 1), (1, 1)))
    wr = np.asarray(weight, np.float32).reshape(GO, GI, KK, LI, LO)
    # wc[(g,l), k*64 + c*8 + m] = wr[c,g,k,l,m]
    wc = np.transpose(wr, (1, 3, 2, 0, 4)).reshape(32, WC).astype(np.float16)
    mask = np.zeros((32, GI), np.float16)
    mask[np.arange(32), np.arange(32) // LI] = 1.0
    out_buf = np.empty((4, GO * LO, HO, WO), np.float32)
    futs = [pool.submit(_run_group, dispatches[g], xp, wc, mask,
                        list(range(g * CORES_PER, (g + 1) * CORES_PER)), out_buf)
            for g in range(N_WAY)]
    for f in futs:
        f.result()
    return out_buf


# revision 11
# speedup vs baseline: 1.2946x; 1.0440x over previous
"""CapsuleConv2d (3x3, s1, p1, L_in=4, L_out=8, 3 routing iters) on 8 trn2 cores.

Sharding: data-parallel over (N=4 images) x (2 half-images of 28 rows) = 8
shards, one per core.

Dispatch is latency-optimized for the axon tunnel (wall-clock is dominated by
host<->device RPC legs, not device compute):
  - inputs shipped compact in fp16: per-core padded x slice (111KB) + the raw
    37KB weight tensor + a 512B block-diag mask; the block-diagonal matmul
    operand (wmm) and the uniform-vote operand (wsum) are expanded ON DEVICE
    (one DVE multiply + one ACT scale), instead of shipping 664KB of
    host-expanded fp32 weights per core
  - output shipped int8 (102KB/core): |v| < 1 is guaranteed by squash, so the
    final squash folds a x127 scale into phi and writes int8 directly; the
    host divides by 127 (quantization adds ~0.004 abs err vs the 2e-2 gate)
  - the work is split into EIGHT independent single-core dispatches driven by
    eight threads; their h2d/exec/d2h RPC chains overlap on the tunnel
    (measured ~20% faster than one 8-core dispatch, whose legs serialize)
  - each jitted shard_map callable is built/compiled ONCE and cached; a call
    issues h2d + exec + d2h fully async and blocks only on the final host
    copies; each thread also builds its own input block and assembles its own
    output chunks, overlapping host prep with the other threads' RPCs

Device kernel (per core, 1568 positions as 14 tiles of 2 rows):
  - PE: priors u via block-diag matmuls (fp16 in, fp32 PSUM); the uniform
    first vote s0 folded into a second accumulating matmul
  - DVE: products, segmented reductions, softmax pieces, squash
  - ACT: PSUM->SBUF copies, exp, sqrt
Per-position free-dim layout for priors u[c,m,k,g]: idx = c*576 + m*72 + k*8 + g
(c = out-capsule 8, m = out-length 8, k = kernel offset 9, g = in-capsule 8).

Engine ISA instructions carry at most one semaphore wait in hardware; building
on bacc.Bacc (not bass.Bass) runs generate_event_semaphores at finalize, which
legalizes the multi-waits Tile emits for cross-engine dependencies.
"""

import numpy as np

import concourse.bass as bass
import concourse.mybir as mybir
import concourse.tile as tile
from concourse import bacc

FP32 = mybir.dt.float32
FP16 = mybir.dt.float16
INT8 = mybir.dt.int8
VSCALE = 127.0   # |v| < 1 always (squash: ||v|| = ||s||^2/(1+||s||^2) < 1)
AF = mybir.ActivationFunctionType
MULT = mybir.AluOpType.mult

KK, GI, GO, LI, LO = 9, 8, 8, 4, 8
HO = WO = 56
ROWS = 28            # output rows per core
SH, SW = ROWS + 2, WO + 2   # 30 x 58 padded input slice per core
TP = 114             # 2 output rows + 2 junk pad positions per tile
NT = 14              # tiles per core (2 rows each)
CM = GO * LO         # 64
CKG = GO * KK * GI   # 576 (c,k,g)
UF = GO * LO * KK * GI  # 4608 (c,m,k,g)
WC = KK * CM         # 576 compact weight columns: (k,c,m)

# free-dim strides in u
SC, SM, SK, SG = 576, 72, 8, 1

XCOLS = SH * SW                  # fp16 x columns per core
INF = XCOLS + WC + GI            # fused input columns: x | wc | mask

N_WAY = 8                        # parallel dispatches
CORES_PER = 1                    # cores per dispatch


def _v(a, dims):
    """Re-view an AP (taken at a tile's origin) with explicit free [step,count] dims."""
    return bass.AP(a.tensor, a.offset, [list(a.ap[0])] + [list(d) for d in dims])


def build_program():
    nc = bacc.Bacc()
    # single fused input (one DMA, one semaphore -> LDWEIGHTS can encode the wait)
    inp = nc.dram_tensor("inp", [32, INF], FP16, kind="ExternalInput")
    out = nc.dram_tensor("out", [NT * TP, CM], INT8, kind="ExternalOutput")

    with tile.TileContext(nc) as tc:
        with (
            tc.tile_pool(name="singles", bufs=1) as singles,
            tc.tile_pool(name="upool", bufs=2) as upool,
            tc.tile_pool(name="ttpool", bufs=4) as ttpool,
            tc.tile_pool(name="mid", bufs=4) as mid,
            tc.tile_pool(name="tiny", bufs=3) as tiny,
            tc.tile_pool(name="vout", bufs=3) as vout,
            tc.tile_pool(name="pu", bufs=3, space="PSUM") as pupool,
            tc.tile_pool(name="ps0", bufs=2, space="PSUM") as ps0pool,
        ):
            inp_sb = singles.tile([32, INF], FP16)
            nc.sync.dma_start(out=inp_sb[:], in_=inp[:])
            xs_flat = inp_sb[:, :XCOLS]
            wc_sb = inp_sb[:, XCOLS:XCOLS + WC]  # [32, (k,c,m)]
            mask_sb = inp_sb[:, XCOLS + WC:]     # [32, g2]: 1 iff g2 == p//4

            # ---- on-device weight expansion ----
            # wsum[(g,l), k*64 + c*8 + m] = wc/9  (same layout as wc)
            wsum_sb = singles.tile([32, WC], FP16)
            nc.scalar.mul(wsum_sb[:], wc_sb, 1.0 / KK)
            # wmm[(g,l), k*512 + (c*8+m)*8 + g2] = wc[(g,l), k*64+c*8+m] * mask[g2]
            wmm_sb = singles.tile([32, KK * 512], FP16)
            nc.vector.tensor_tensor(
                _v(wmm_sb[:], [[512, KK], [8, CM], [1, GI]]),
                _v(wc_sb, [[CM, KK], [1, CM], [0, GI]]),
                _v(mask_sb, [[0, KK], [0, CM], [1, GI]]),
                op=MULT)

            for t in range(NT):
                h0 = 2 * t
                # ---- priors: u[pos; c,m,k,g] and s0[pos; c,m] on PE ----
                u = upool.tile([TP, UF], FP32)
                ps0 = ps0pool.tile([TP, CM], FP32)
                for k in range(KK):
                    di, dj = k // 3, k % 3
                    # flat 114-run covering 2 rows of 56 (+2 junk at 56,57):
                    # LDWEIGHTS needs a single-free-dim AP
                    o = (h0 + di) * SW + dj
                    lhsT = xs_flat[:, o:o + TP]  # [32, 114] fp16
                    pu = pupool.tile([TP, 512], FP32)
                    nc.tensor.matmul(pu[:], lhsT, wmm_sb[:, k * 512:(k + 1) * 512],
                                     start=True, stop=True)
                    nc.tensor.matmul(ps0[:], lhsT, wsum_sb[:, k * CM:(k + 1) * CM],
                                     start=(k == 0), stop=(k == KK - 1))
                    # psum (c,m,g) -> sbuf u[:, c,m,k=k,g]  (strided write, ACT)
                    u4 = _v(u[:], [[SC, GO], [SM, LO], [SK, KK], [SG, GI]])
                    nc.scalar.copy(out=u4[:, :, :, k, :], in_=pu[:])

                # ---- routing ----
                def squash(s_ap, vdst, scale=None):
                    sq = tiny.tile([TP, CM], FP32, tag="sq")
                    nc.vector.tensor_mul(sq[:], s_ap, s_ap)
                    n2 = tiny.tile([TP, GO], FP32, tag="n2")
                    nc.vector.reduce_sum(n2[:], _v(sq[:], [[LO, GO], [1, LO]]),
                                         axis=mybir.AxisListType.X)
                    rt = tiny.tile([TP, GO], FP32, tag="rt")
                    nc.scalar.activation(rt[:], n2[:], AF.Sqrt)
                    n2p1 = tiny.tile([TP, GO], FP32, tag="n2p1")
                    nc.scalar.add(n2p1[:], n2[:], 1.0)
                    inv = tiny.tile([TP, GO], FP32, tag="inv")
                    nc.vector.reciprocal(inv[:], n2p1[:])
                    phi = tiny.tile([TP, GO], FP32, tag="phi")
                    nc.vector.tensor_mul(phi[:], rt[:], inv[:])
                    if scale is not None:
                        phis = tiny.tile([TP, GO], FP32, tag="phis")
                        nc.scalar.mul(phis[:], phi[:], scale)
                        phi = phis
                    # v = s * phi (phi broadcast over m)
                    return nc.vector.tensor_tensor(
                        _v(vdst[:], [[LO, GO], [1, LO]]),
                        bass.AP(s_ap.tensor, s_ap.offset,
                                [list(s_ap.ap[0]), [LO, GO], [1, LO]]),
                        _v(phi[:], [[1, GO], [0, LO]]),
                        op=MULT)

                s0 = tiny.tile([TP, CM], FP32, tag="s0")
                nc.scalar.copy(out=s0[:], in_=ps0[:])
                v = vout.tile([TP, CM], FP32, tag="v")
                squash(s0[:], v)

                b_prev = None
                for r in (1, 2):
                    # tt = u * v  (v[c,m] broadcast over k,g)
                    tt = ttpool.tile([TP, UF], FP32, tag="tt")
                    nc.vector.tensor_tensor(
                        _v(tt[:], [[SC, GO], [SM, LO], [1, KK * GI]]),
                        _v(u[:], [[SC, GO], [SM, LO], [1, KK * GI]]),
                        _v(v[:], [[LO, GO], [1, LO], [0, KK * GI]]),
                        op=MULT)
                    # b = sum_m tt  -> [pos; c,k,g]
                    b = mid.tile([TP, CKG], FP32, tag="b")
                    nc.vector.reduce_sum(
                        b[:], _v(tt[:], [[SC, GO], [SK, KK], [SG, GI], [SM, LO]]),
                        axis=mybir.AxisListType.X)
                    if b_prev is not None:
                        nc.vector.tensor_add(b[:], b[:], b_prev[:])
                    b_prev = b
                    # softmax over k (segments of the c,k,g layout)
                    e = mid.tile([TP, CKG], FP32, tag="e")
                    nc.scalar.activation(e[:], b[:], AF.Exp)
                    ssum = tiny.tile([TP, CM], FP32, tag="ssum")
                    nc.vector.reduce_sum(
                        ssum[:], _v(e[:], [[KK * GI, GO], [SG, GI], [SK, KK]]),
                        axis=mybir.AxisListType.X)
                    invs = tiny.tile([TP, CM], FP32, tag="invs")
                    nc.vector.reciprocal(invs[:], ssum[:])
                    p = mid.tile([TP, CKG], FP32, tag="p")
                    nc.vector.tensor_tensor(
                        _v(p[:], [[KK * GI, GO], [SK, KK], [SG, GI]]),
                        _v(e[:], [[KK * GI, GO], [SK, KK], [SG, GI]]),
                        _v(invs[:], [[GI, GO], [0, KK], [1, GI]]),
                        op=MULT)
                    # tt2 = p * u ; s = sum_{k,g} tt2
                    tt2 = ttpool.tile([TP, UF], FP32, tag="tt")
                    nc.vector.tensor_tensor(
                        _v(tt2[:], [[SC, GO], [SM, LO], [SK, KK], [SG, GI]]),
                        _v(u[:], [[SC, GO], [SM, LO], [SK, KK], [SG, GI]]),
                        _v(p[:], [[KK * GI, GO], [0, LO], [SK, KK], [SG, GI]]),
                        op=MULT)
                    s = tiny.tile([TP, CM], FP32, tag="s")
                    nc.vector.reduce_sum(
                        s[:], _v(tt2[:], [[SC, GO], [SM, LO], [SK, KK], [SG, GI]]),
                        axis=mybir.AxisListType.XY)
                    # final iteration writes scaled int8 directly (DMA'd out)
                    if r == 2:
                        v = vout.tile([TP, CM], INT8, tag="v8")
                        squash(s[:], v, scale=VSCALE)
                    else:
                        v = vout.tile([TP, CM], FP32, tag="v")
                        squash(s[:], v)

                nc.sync.dma_start(out=out[t * TP:(t + 1) * TP, :], in_=v[:])
    return nc


# ---------------- host side ----------------

_STATE = None

# (image, top row) of each 28-row chunk; chunk i runs on global core i
_CHUNKS = [(n, h0) for n in range(4) for h0 in (0, ROWS)]


def _get_state():
    """Build the program and the cached jitted callables (one per 2-core
    dispatch group) once."""
    global _STATE
    if _STATE is None:
        import jax
        import concurrent.futures as cf
        from jax.sharding import Mesh, PartitionSpec
        from jax.experimental.shard_map import shard_map
        from concourse.bass2jax import (_bass_exec_p, install_neuronx_cc_hook,
                                        partition_id_tensor)

        nc = build_program()
        nc.finalize()
        install_neuronx_cc_hook()

        partition_name = (nc.partition_id_tensor.name
                          if nc.partition_id_tensor else None)
        in_names, out_names, out_avals = [], [], []
        for alloc in nc.m.functions[0].allocations:
            if not isinstance(alloc, mybir.MemoryLocationSet):
                continue
            name = alloc.memorylocations[0].name
            if alloc.kind == "ExternalInput":
                if name != partition_name:
                    in_names.append(name)
            elif alloc.kind == "ExternalOutput":
                out_names.append(name)
                out_avals.append(jax.core.ShapedArray(
                    tuple(alloc.tensor_shape), mybir.dt.np(alloc.dtype)))
        all_names = list(in_names)
        if partition_name is not None:
            all_names.append(partition_name)

        def _body(*args):
            operands = list(args)
            if partition_name is not None:
                operands.append(partition_id_tensor())
            return tuple(_bass_exec_p.bind(
                *operands,
                out_avals=tuple(out_avals), in_names=tuple(all_names),
                out_names=tuple(out_names), lowering_input_output_aliases=(),
                sim_require_finite=True, sim_require_nnan=True, nc=nc))

        devices = jax.devices()[:N_WAY * CORES_PER]
        sample = np.zeros((CORES_PER * 32, INF), np.float16)
        dispatches = []
        for g in range(N_WAY):
            mesh = Mesh(np.asarray(devices[g * CORES_PER:(g + 1) * CORES_PER]),
                        ("core",))
            jitted = jax.jit(
                shard_map(_body, mesh=mesh,
                          in_specs=(PartitionSpec("core"),) * len(in_names),
                          out_specs=(PartitionSpec("core"),) * len(out_names),
                          check_rep=False),
                keep_unused=True)
            # AOT-compile: direct Compiled calls skip the jit dispatch layers
            dispatches.append(jitted.lower(sample).compile())
        pool = cf.ThreadPoolExecutor(N_WAY)
        _STATE = (dispatches, pool)
    return _STATE


def _run_group(dispatch, x, wc, mask, cores, out_buf):
    """Build this group's input block (pad + fp16 cast of its x slice),
    dispatch it, fetch, and assemble its chunks into out_buf (all inside the
    worker thread so host work overlaps the other groups' RPC legs)."""
    blk = np.zeros((len(cores) * 32, INF), np.float16)
    for i, core in enumerate(cores):
        n, h0 = _CHUNKS[core]
        rows = blk[i * 32:(i + 1) * 32]
        xsl = rows[:, :XCOLS].reshape(32, SH, SW)
        # rows j of the padded slice map to x rows h0+j-1; borders stay zero
        a = max(h0 - 1, 0)
        b = min(h0 + SH - 1, HO)
        xsl[:, a - (h0 - 1):b - (h0 - 1), 1:1 + WO] = x[n, :, a:b, :]
        rows[:, XCOLS:XCOLS + WC] = wc
        rows[:, XCOLS + WC:] = mask
    out = dispatch(blk)[0]
    shards = sorted(out.addressable_shards, key=lambda s: s.index[0].start or 0)
    for s in shards:
        s.data.copy_to_host_async()
    for i, core in enumerate(cores):
        o = np.asarray(shards[i].data, np.float32).reshape(NT, TP, CM)
        o *= 1.0 / VSCALE
        # TP=114 run: [0:56] = row 0, [58:114] = row 1, 56/57 junk
        o = np.stack([o[:, :WO], o[:, SW:SW + WO]], axis=1).reshape(ROWS, WO, CM)
        n, h0 = _CHUNKS[core]
        out_buf[n, :, h0:h0 + ROWS, :] = np.transpose(o, (2, 0, 1))


def kernel(x, weight):
    dispatches, pool = _get_state()
    x = np.asarray(x)
    wr = np.asarray(weight, np.float32).reshape(GO, GI, KK, LI, LO)
    # wc[(g,l), k*64 + c*8 + m] = wr[c,g,k,l,m]
    wc = np.transpose(wr, (1, 3, 2, 0, 4)).reshape(32, WC).astype(np.float16)
    mask = np.zeros((32, GI), np.float16)
    mask[np.arange(32), np.arange(32) // LI] = 1.0
    out_buf = np.empty((4, GO * LO, HO, WO), np.float32)
    futs = [pool.submit(_run_group, dispatches[g], x, wc, mask,
                        list(range(g * CORES_PER, (g + 1) * CORES_PER)), out_buf)
            for g in range(N_WAY)]
    for f in futs:
        f.result()
    return out_buf
